# revision 1
# baseline (speedup 1.0000x reference)
"""Trainium2 Bass kernel for BatchedExpertMoEDispatch.

Strategy (expert-parallel, sparse dispatch):
  - Host computes the routing table from (expert_ids, expert_weights):
    for each expert e the unique token list idx_e and combined coefficient
    coeff_e (duplicate (token, expert) slots merge by summing weights).
  - The token groups are "all-to-all"ed host-side (full-I/O contract): core e
    receives x.T gathered to its tokens [H, NCAP], its expert's gate/up/down
    weights in natural layout, and coeff_e.
  - Each core runs the full FFN for its expert on its tokens:
        gT = Wg.T @ xT ; uT = Wu.T @ xT          (PSUM, fp32 accum)
        hT = silu(gT) * uT                        (ACT + DVE)
        yT = Wd.T @ hT                            (PSUM, fp32 accum)
        outT = yT * coeff (broadcast over partitions)
    All activations live feature-major so every matmul operand is natural
    layout; matmuls run in float32r (full PE rate, fp32 I/O).
  - Host scatter-adds each core's outT back: out[idx_e] += outT[:, :n_e].T.

Capacity: NCAP tokens/core/round.  If any expert has more assigned tokens
(possible for adversarial routing distributions), the same compiled program
runs additional rounds on the remainder.
"""

import os
import sys

import numpy as np

for _p in ("/opt/trn_rl_repo", "/root/.axon_site/_ro/trn_rl_repo"):
    if os.path.isdir(_p) and _p not in sys.path:
        sys.path.append(_p)

import concourse.bacc as bacc
import concourse.mybir as mybir
import concourse.tile as tile
from concourse.bass_utils import run_bass_kernel_spmd

# Problem shapes (hardcoded per contract).
T, H, F, E, K = 4096, 1024, 2048, 8, 2
NCORES = 8
CKS = [512, 472]     # moving-operand chunks (fp32 max 512)
NCAP = sum(CKS)      # token capacity per core per round (>= seed-wise max)
NCHUNK = len(CKS)
COFF = [0, 512]      # chunk offsets
KH = H // 128        # 8  k-tiles over H
KF = F // 128        # 16 k-tiles over F
FP32 = mybir.dt.float32
FP32R = mybir.dt.float32r
MUL = mybir.AluOpType.mult

_PROGRAM = None

# Extra kwargs for run_bass_kernel_spmd — test harness pokes this to enable
# tracing; the grader path leaves it empty.
RUN_KWARGS: dict = {}
LAST_RESULTS = []


def build_program():
    """Build + compile the per-core SPMD FFN program (shared by all cores)."""
    nc = bacc.Bacc(
        "TRN2", target_bir_lowering=False, debug=False, num_devices=NCORES
    )
    xt_d = nc.dram_tensor("xt", [H, NCAP], FP32R, kind="ExternalInput")
    wg_d = nc.dram_tensor("wg", [H, F], FP32R, kind="ExternalInput")
    wu_d = nc.dram_tensor("wu", [H, F], FP32R, kind="ExternalInput")
    wd_d = nc.dram_tensor("wd", [F, H], FP32R, kind="ExternalInput")
    cf_d = nc.dram_tensor("cf", [1, NCAP], FP32, kind="ExternalInput")
    yt_d = nc.dram_tensor("yt", [H, NCAP], FP32, kind="ExternalOutput")

    with tile.TileContext(nc) as tc:
        from contextlib import ExitStack

        with ExitStack() as ctx:
            xt_pool = ctx.enter_context(tc.tile_pool(name="xt", bufs=1))
            ht_pool = ctx.enter_context(tc.tile_pool(name="ht", bufs=1))
            cf_pool = ctx.enter_context(tc.tile_pool(name="cf", bufs=1))
            wg_pool = ctx.enter_context(tc.tile_pool(name="wg", bufs=4))
            wu_pool = ctx.enter_context(tc.tile_pool(name="wu", bufs=4))
            wd_pool = ctx.enter_context(tc.tile_pool(name="wd", bufs=3))
            sl_pool = ctx.enter_context(tc.tile_pool(name="sl", bufs=4))
            ob_pool = ctx.enter_context(tc.tile_pool(name="ob", bufs=4))
            pg_pool = ctx.enter_context(tc.tile_pool(name="pg", bufs=3, space="PSUM"))
            pu_pool = ctx.enter_context(tc.tile_pool(name="pu", bufs=3, space="PSUM"))
            py_pool = ctx.enter_context(tc.tile_pool(name="py", bufs=2, space="PSUM"))

            # coeff, broadcast to all 128 partitions (needed only in phase 2;
            # issue on gpsimd's SWDGE queue to keep sync free for weights)
            cf_t = cf_pool.tile([128, NCAP], FP32, tag="cf")
            nc.gpsimd.dma_start(cf_t[:], cf_d.ap().partition_broadcast(128))

            # gate/up weight column loader: [128, KH*128], k-major free dim
            wgwu = {}

            def load_wgwu(f):
                wgt = wg_pool.tile([128, KH * 128], FP32R, tag="wg")
                wut = wu_pool.tile([128, KH * 128], FP32R, tag="wu")
                src_g = wg_d.ap()[:, f * 128 : (f + 1) * 128].rearrange(
                    "(k p) m -> p k m", p=128
                )
                src_u = wu_d.ap()[:, f * 128 : (f + 1) * 128].rearrange(
                    "(k p) m -> p k m", p=128
                )
                nc.sync.dma_start(
                    wgt[:].rearrange("p (k m) -> p k m", m=128), src_g
                )
                nc.sync.dma_start(
                    wut[:].rearrange("p (k m) -> p k m", m=128), src_u
                )
                wgwu[f] = (wgt, wut)

            # xT: per chunk, two k-half tiles [128, 4*ck] (k-major).
            # Sync-queue order = critical-first: xtc0 half 0, f0 weights,
            # xtc0 half 1, xtc1 halves, then the remaining weight columns.
            # One queue so nothing non-critical competes for HBM during
            # startup; the first matmul burst needs only xtc0h0 + wg0.
            xtc = {}

            def load_xt_half(ci, h):
                ck = CKS[ci]
                cs = COFF[ci]
                t = xt_pool.tile([128, 4 * ck], FP32R, tag=f"xtc{ci}_{h}")
                src = xt_d.ap()[
                    h * 512 : (h + 1) * 512, cs : cs + ck
                ].rearrange("(k p) t -> p k t", p=128)
                nc.sync.dma_start(
                    t[:].rearrange("p (k t) -> p k t", t=ck), src
                )
                xtc.setdefault(ci, []).append(t)

            load_xt_half(0, 0)
            load_wgwu(0)
            load_xt_half(0, 1)
            load_xt_half(1, 0)
            load_xt_half(1, 1)

            # Phase 1: hT[f] = silu(Wg[:,f].T @ xT) * (Wu[:,f].T @ xT)
            hts = []
            for f in range(KF):
                if f not in wgwu:
                    load_wgwu(f)
                wgt, wut = wgwu[f]
                ht = ht_pool.tile([128, NCAP], FP32R, tag=f"ht{f}")
                for ci in range(NCHUNK):
                    ck = CKS[ci]
                    cs, ce = COFF[ci], COFF[ci] + ck
                    pg = pg_pool.tile([128, ck], FP32, tag="pg")
                    pu = pu_pool.tile([128, ck], FP32, tag="pu")
                    # For the very first f-tile, interleave g/u in k-halves so
                    # the first burst only needs the first xT half-tile (the
                    # startup DMA stall shrinks by ~1MB of transfer time).
                    if f == 0 and ci == 0:
                        k_bursts = [(0, 4), (4, 8)]
                    else:
                        k_bursts = [(0, KH)]
                    for lo, hi in k_bursts:
                        for dst, w in ((pg, wgt), (pu, wut)):
                            for k in range(lo, hi):
                                nc.tensor.matmul(
                                    dst[:],
                                    w[:, k * 128 : (k + 1) * 128],
                                    xtc[ci][k // 4][
                                        :, (k % 4) * ck : (k % 4 + 1) * ck
                                    ],
                                    start=(k == 0),
                                    stop=(k == KH - 1),
                                )
                    sl = sl_pool.tile([128, ck], FP32, tag="sl")
                    nc.scalar.activation(
                        sl[:], pg[:], mybir.ActivationFunctionType.Sigmoid
                    )
                    nc.vector.tensor_tensor(sl[:], sl[:], pg[:], MUL)
                    nc.vector.tensor_tensor(ht[:, cs:ce], sl[:], pu[:], MUL)
                hts.append(ht)

            # Phase 2: yT[j] = Wd[:,j].T @ hT, scaled by coeff
            for j in range(KH):
                wdt = wd_pool.tile([128, KF * 128], FP32R, tag="wd")
                src_d = (
                    wd_d.ap()[:, j * 128 : (j + 1) * 128]
                    .rearrange("(k p) m -> p k m", p=128)
                )
                nc.sync.dma_start(
                    wdt[:].rearrange("p (k m) -> p k m", m=128), src_d
                )
                for ci in range(NCHUNK):
                    ck = CKS[ci]
                    cs, ce = COFF[ci], COFF[ci] + ck
                    py = py_pool.tile([128, ck], FP32, tag="py")
                    for kf in range(KF):
                        nc.tensor.matmul(
                            py[:],
                            wdt[:, kf * 128 : (kf + 1) * 128],
                            hts[kf][:, cs:ce],
                            start=(kf == 0),
                            stop=(kf == KF - 1),
                        )
                    # Final unit: split the coeff-mul + store into halves so
                    # the last DMA starts while the second half multiplies
                    # (shortens the kernel-end critical path slightly).
                    nsplit = 2 if (j == KH - 1 and ci == NCHUNK - 1) else 1
                    ob = ob_pool.tile([128, ck], FP32, tag="ob")
                    hw = ck // nsplit
                    for s in range(nsplit):
                        lo, hi = s * hw, (s + 1) * hw
                        nc.vector.tensor_tensor(
                            ob[:, lo:hi], py[:, lo:hi], cf_t[:, cs + lo : cs + hi], MUL
                        )
                        nc.scalar.dma_start(
                            yt_d.ap()[j * 128 : (j + 1) * 128, cs + lo : cs + hi],
                            ob[:, lo:hi],
                        )

    nc.compile()
    return nc


def _get_program():
    global _PROGRAM
    if _PROGRAM is None:
        _PROGRAM = build_program()
    return _PROGRAM


def kernel(x, expert_ids, expert_weights, gate_weights, up_weights, down_weights):
    x = np.ascontiguousarray(np.asarray(x, dtype=np.float32))
    expert_ids = np.asarray(expert_ids)
    expert_weights = np.asarray(expert_weights, dtype=np.float32)
    gate_weights = np.ascontiguousarray(np.asarray(gate_weights, dtype=np.float32))
    up_weights = np.ascontiguousarray(np.asarray(up_weights, dtype=np.float32))
    down_weights = np.ascontiguousarray(np.asarray(down_weights, dtype=np.float32))

    t_dim, h_dim = x.shape
    n_exp = gate_weights.shape[0]
    assert h_dim == H and gate_weights.shape[1:] == (H, F), (
        "program compiled for H=1024, F=2048"
    )
    assert n_exp == NCORES, "expert-parallel mapping assumes E == 8 cores"

    # Routing table: per-token combined coefficient per expert.
    coeff = np.zeros((t_dim, n_exp), np.float32)
    rows = np.arange(t_dim)
    for k in range(expert_ids.shape[1]):
        np.add.at(coeff, (rows, expert_ids[:, k]), expert_weights[:, k])

    idx_per_e = [np.nonzero(coeff[:, e])[0] for e in range(n_exp)]
    rounds = max(1, max((len(i) + NCAP - 1) // NCAP for i in idx_per_e))

    xT = np.ascontiguousarray(x.T)  # [H, T]
    nc = _get_program()

    out = np.zeros((t_dim, h_dim), np.float32)
    LAST_RESULTS.clear()
    for r in range(rounds):
        in_maps = []
        idx_r_per_e = []
        for e in range(n_exp):
            idx_r = idx_per_e[e][r * NCAP : (r + 1) * NCAP]
            idx_r_per_e.append(idx_r)
            xte = np.zeros((h_dim, NCAP), np.float32)
            cfe = np.zeros((1, NCAP), np.float32)
            if len(idx_r):
                xte[:, : len(idx_r)] = xT[:, idx_r]
                cfe[0, : len(idx_r)] = coeff[idx_r, e]
            in_maps.append(
                {
                    "xt": xte,
                    "wg": gate_weights[e],
                    "wu": up_weights[e],
                    "wd": down_weights[e],
                    "cf": cfe,
                }
            )
        res = run_bass_kernel_spmd(
            nc, in_maps, core_ids=list(range(NCORES)), **RUN_KWARGS
        )
        LAST_RESULTS.append(res)
        for e in range(n_exp):
            idx_r = idx_r_per_e[e]
            if len(idx_r):
                yt = res.results[e]["yt"]  # [H, NCAP], already coeff-scaled
                out[idx_r, :] += yt[:, : len(idx_r)].T
    return out



# revision 5
# speedup vs baseline: 1.0647x; 1.0647x over previous
"""Trainium2 Bass kernel for BatchedExpertMoEDispatch.

Strategy (expert-parallel, sparse dispatch, bf16 compute):
  - Host computes the routing table from (expert_ids, expert_weights):
    for each expert e the unique token list idx_e and combined coefficient
    coeff_e (duplicate (token, expert) slots merge by summing weights).
  - The token groups are "all-to-all"ed host-side (full-I/O contract): core e
    receives its expert's tokens and weights pre-packed in partition-major
    bf16 layouts so every DMA is a straight contiguous copy (2-4KB
    per-partition lines; the fp32 rearrange loads of the previous version
    were 512B-descriptor-bound and stalled startup by ~13us).
  - Each core runs the full FFN for its expert on its tokens:
        gT = Wg.T @ xT ; uT = Wu.T @ xT          (bf16 matmul, fp32 PSUM)
        hT = silu(gT) * uT                        (ACT Silu + DVE, bf16 out)
        yT = Wd.T @ hT                            (bf16 matmul, fp32 PSUM)
        outT = yT * coeff (broadcast over partitions)
    bf16 matmuls run at the same 1 row/cycle as float32r but qualify for
    fast-weight-load + background LDWEIGHTS pull-ahead, so the per-matmul
    weight-load overhead of the fp32r version (~12%) disappears.  Stationary
    weight tiles are reused across both token chunks (chunk-inner loop) to
    halve LDWEIGHTS traffic.
  - Host scatter-adds each core's outT back: out[idx_e] += outT[:, :n_e].T.

Capacity: NCAP tokens/core/round.  If any expert has more assigned tokens,
the same compiled program runs additional rounds on the remainder.
"""

import os
import sys

import numpy as np
import ml_dtypes

for _p in ("/opt/trn_rl_repo", "/root/.axon_site/_ro/trn_rl_repo"):
    if os.path.isdir(_p) and _p not in sys.path:
        sys.path.append(_p)

import concourse.bacc as bacc
import concourse.mybir as mybir
import concourse.tile as tile
from concourse.bass_utils import run_bass_kernel_spmd

# Problem shapes (hardcoded per contract).
T, H, F, E, K = 4096, 1024, 2048, 8, 2
NCORES = 8
CKS = [512, 472]     # token chunks (PSUM bank = 512 fp32)
NCAP = sum(CKS)      # token capacity per core per round (>= seed-wise max)
COFF = [0, 512]      # chunk offsets
KH = H // 128        # 8  k-tiles over H
KF = F // 128        # 16 k-tiles over F
FP32 = mybir.dt.float32
BF16 = mybir.dt.bfloat16
NPBF16 = ml_dtypes.bfloat16
MUL = mybir.AluOpType.mult

_PROGRAM = None

# Extra kwargs for run_bass_kernel_spmd — test harness pokes this to enable
# tracing; the grader path leaves it empty.
RUN_KWARGS: dict = {}
LAST_RESULTS = []


def build_program():
    """Build + compile the per-core SPMD FFN program (shared by all cores)."""
    nc = bacc.Bacc(
        "TRN2", target_bir_lowering=False, debug=False, num_devices=NCORES
    )
    # Packed layouts (host-side prep):
    #   xp[p, k*NCAP+t] = x[idx[t], k*128+p]
    #   wg/wu[p, f*1024 + k*128 + m] = W[k*128+p, f*128+m]
    #   wd[p, j*2048 + kf*128 + m]   = Wd[kf*128+p, j*128+m]
    xp_d = nc.dram_tensor("xp", [128, KH * NCAP], BF16, kind="ExternalInput")
    wg_d = nc.dram_tensor("wg", [128, KF * KH * 128], BF16, kind="ExternalInput")
    wu_d = nc.dram_tensor("wu", [128, KF * KH * 128], BF16, kind="ExternalInput")
    wd_d = nc.dram_tensor("wd", [128, KH * KF * 128], BF16, kind="ExternalInput")
    cf_d = nc.dram_tensor("cf", [1, NCAP], FP32, kind="ExternalInput")
    yt_d = nc.dram_tensor("yt", [H, NCAP], FP32, kind="ExternalOutput")

    with tile.TileContext(nc) as tc:
        from contextlib import ExitStack

        with ExitStack() as ctx:
            xk_pool = ctx.enter_context(tc.tile_pool(name="xk", bufs=KH))
            wg_pool = ctx.enter_context(tc.tile_pool(name="wg", bufs=KF))
            wu_pool = ctx.enter_context(tc.tile_pool(name="wu", bufs=KF))
            wd_pool = ctx.enter_context(tc.tile_pool(name="wd", bufs=KH))
            ht_pool = ctx.enter_context(tc.tile_pool(name="ht", bufs=KF))
            cf_pool = ctx.enter_context(tc.tile_pool(name="cf", bufs=1))
            sl_pool = ctx.enter_context(tc.tile_pool(name="sl", bufs=4))
            ob_pool = ctx.enter_context(tc.tile_pool(name="ob", bufs=4))
            pg_pool = ctx.enter_context(tc.tile_pool(name="pg", bufs=3, space="PSUM"))
            pu_pool = ctx.enter_context(tc.tile_pool(name="pu", bufs=3, space="PSUM"))
            py_pool = ctx.enter_context(tc.tile_pool(name="py", bufs=2, space="PSUM"))

            # coeff, broadcast to all 128 partitions (needed only in phase 2;
            # issue on gpsimd's SWDGE queue to keep sync free for weights)
            cf_t = cf_pool.tile([128, NCAP], FP32, tag="cf")
            nc.gpsimd.dma_start(cf_t[:], cf_d.ap().partition_broadcast(128))

            xks = {}

            def load_xk(k):
                t = xk_pool.tile([128, NCAP], BF16, tag="xk")
                nc.sync.dma_start(t[:], xp_d.ap()[:, k * NCAP : (k + 1) * NCAP])
                xks[k] = t

            wgwu = {}

            def load_wgwu(f):
                wgt = wg_pool.tile([128, KH * 128], BF16, tag="wg")
                wut = wu_pool.tile([128, KH * 128], BF16, tag="wu")
                nc.sync.dma_start(
                    wgt[:], wg_d.ap()[:, f * KH * 128 : (f + 1) * KH * 128]
                )
                nc.sync.dma_start(
                    wut[:], wu_d.ap()[:, f * KH * 128 : (f + 1) * KH * 128]
                )
                wgwu[f] = (wgt, wut)

            wds = {}

            def load_wd(j):
                t = wd_pool.tile([128, KF * 128], BF16, tag="wd")
                nc.sync.dma_start(t[:], wd_d.ap()[:, j * KF * 128 : (j + 1) * KF * 128])
                wds[j] = t

            # Critical-first DMA order: the f0 gate pass consumes x k-slices
            # 0..7 within its first ~3.3us of matmuls, so front-load them.
            load_xk(0)
            load_wgwu(0)
            load_xk(1)
            load_xk(2)
            load_xk(3)
            load_wgwu(1)
            for k in range(4, KH):
                load_xk(k)

            # Phase 1: hT[f] = silu(Wg[:,f].T @ xT) * (Wu[:,f].T @ xT)
            hts = []
            for f in range(KF):
                if f not in wgwu:
                    load_wgwu(f)
                # prefetch a couple of f-columns ahead; down weights at the end
                pf = f + 2
                if pf < KF and pf not in wgwu:
                    load_wgwu(pf)
                if f == KF - 1:
                    for j in range(KH):
                        load_wd(j)
                wgt, wut = wgwu[f]
                ht = ht_pool.tile([128, NCAP], BF16, tag="ht")
                pgs, pus = [], []
                for ci in range(2):
                    pgs.append(
                        pg_pool.tile(
                            [128, CKS[ci]], FP32, tag="pg", name=f"pg{ci}"
                        )
                    )
                    pus.append(
                        pu_pool.tile(
                            [128, CKS[ci]], FP32, tag="pu", name=f"pu{ci}"
                        )
                    )
                if f == 0:
                    # k-outer: consume each x k-slice for gate AND up before
                    # needing the next — halves the startup DMA arrival rate
                    # the first accumulation pass demands.
                    order = [
                        (dsts, w, k)
                        for k in range(KH)
                        for dsts, w in ((pgs, wgt), (pus, wut))
                    ]
                else:
                    order = [
                        (dsts, w, k)
                        for dsts, w in ((pgs, wgt), (pus, wut))
                        for k in range(KH)
                    ]
                for dsts, w, k in order:
                    for ci in range(2):
                        cs = COFF[ci]
                        ck = CKS[ci]
                        nc.tensor.matmul(
                            dsts[ci][:],
                            w[:, k * 128 : (k + 1) * 128],
                            xks[k][:, cs : cs + ck],
                            start=(k == 0),
                            stop=(k == KH - 1),
                        )
                for ci in range(2):
                    cs, ck = COFF[ci], CKS[ci]
                    sl = sl_pool.tile([128, ck], FP32, tag="sl")
                    nc.scalar.activation(
                        sl[:], pgs[ci][:], mybir.ActivationFunctionType.Silu
                    )
                    nc.vector.tensor_tensor(
                        ht[:, cs : cs + ck], sl[:], pus[ci][:], MUL
                    )
                hts.append(ht)

            # Phase 2: yT[j] = (Wd[:,j].T @ hT) * coeff
            for j in range(KH):
                wdt = wds[j]
                pys = []
                for ci in range(2):
                    pys.append(
                        py_pool.tile(
                            [128, CKS[ci]], FP32, tag="py", name=f"py{ci}"
                        )
                    )
                for kf in range(KF):
                    for ci in range(2):
                        cs, ck = COFF[ci], CKS[ci]
                        nc.tensor.matmul(
                            pys[ci][:],
                            wdt[:, kf * 128 : (kf + 1) * 128],
                            hts[kf][:, cs : cs + ck],
                            start=(kf == 0),
                            stop=(kf == KF - 1),
                        )
                for ci in range(2):
                    cs, ck = COFF[ci], CKS[ci]
                    # Final unit: split the coeff-mul + store into halves so
                    # the last DMA starts while the second half multiplies.
                    nsplit = 2 if (j == KH - 1 and ci == 1) else 1
                    ob = ob_pool.tile([128, ck], FP32, tag="ob")
                    hw = ck // nsplit
                    for s in range(nsplit):
                        lo, hi = s * hw, (s + 1) * hw if s < nsplit - 1 else ck
                        nc.vector.tensor_tensor(
                            ob[:, lo:hi],
                            pys[ci][:, lo:hi],
                            cf_t[:, cs + lo : cs + hi],
                            MUL,
                        )
                        nc.scalar.dma_start(
                            yt_d.ap()[j * 128 : (j + 1) * 128, cs + lo : cs + hi],
                            ob[:, lo:hi],
                        )

    nc.compile()
    return nc


def _get_program():
    global _PROGRAM
    if _PROGRAM is None:
        _PROGRAM = build_program()
    return _PROGRAM


def _pack_weights(gate_weights, up_weights, down_weights):
    """Pre-pack per-expert weights into partition-major bf16 DMA images."""
    wg_p, wu_p, wd_p = [], [], []
    for e in range(NCORES):
        wg = np.ascontiguousarray(
            gate_weights[e]
            .astype(NPBF16)
            .reshape(KH, 128, KF, 128)
            .transpose(1, 2, 0, 3)
            .reshape(128, KF * KH * 128)
        )
        wu = np.ascontiguousarray(
            up_weights[e]
            .astype(NPBF16)
            .reshape(KH, 128, KF, 128)
            .transpose(1, 2, 0, 3)
            .reshape(128, KF * KH * 128)
        )
        wd = np.ascontiguousarray(
            down_weights[e]
            .astype(NPBF16)
            .reshape(KF, 128, KH, 128)
            .transpose(1, 2, 0, 3)
            .reshape(128, KH * KF * 128)
        )
        wg_p.append(wg)
        wu_p.append(wu)
        wd_p.append(wd)
    return wg_p, wu_p, wd_p


def kernel(x, expert_ids, expert_weights, gate_weights, up_weights, down_weights):
    x = np.ascontiguousarray(np.asarray(x, dtype=np.float32))
    expert_ids = np.asarray(expert_ids)
    expert_weights = np.asarray(expert_weights, dtype=np.float32)
    gate_weights = np.asarray(gate_weights, dtype=np.float32)
    up_weights = np.asarray(up_weights, dtype=np.float32)
    down_weights = np.asarray(down_weights, dtype=np.float32)

    t_dim, h_dim = x.shape
    n_exp = gate_weights.shape[0]
    assert h_dim == H and gate_weights.shape[1:] == (H, F), (
        "program compiled for H=1024, F=2048"
    )
    assert n_exp == NCORES, "expert-parallel mapping assumes E == 8 cores"

    # Routing table: per-token combined coefficient per expert.
    coeff = np.zeros((t_dim, n_exp), np.float32)
    rows = np.arange(t_dim)
    for k in range(expert_ids.shape[1]):
        np.add.at(coeff, (rows, expert_ids[:, k]), expert_weights[:, k])

    idx_per_e = [np.nonzero(coeff[:, e])[0] for e in range(n_exp)]
    rounds = max(1, max((len(i) + NCAP - 1) // NCAP for i in idx_per_e))

    wg_p, wu_p, wd_p = _pack_weights(gate_weights, up_weights, down_weights)
    x16 = x.astype(NPBF16)
    nc = _get_program()

    out = np.zeros((t_dim, h_dim), np.float32)
    LAST_RESULTS.clear()
    for r in range(rounds):
        in_maps = []
        idx_r_per_e = []
        for e in range(n_exp):
            idx_r = idx_per_e[e][r * NCAP : (r + 1) * NCAP]
            idx_r_per_e.append(idx_r)
            xpe = np.zeros((128, KH, NCAP), NPBF16)
            cfe = np.zeros((1, NCAP), np.float32)
            if len(idx_r):
                # [p, k, t] = x[idx[t], k*128+p]
                xpe[:, :, : len(idx_r)] = x16[idx_r].reshape(
                    len(idx_r), KH, 128
                ).transpose(2, 1, 0)
                cfe[0, : len(idx_r)] = coeff[idx_r, e]
            in_maps.append(
                {
                    "xp": xpe.reshape(128, KH * NCAP),
                    "wg": wg_p[e],
                    "wu": wu_p[e],
                    "wd": wd_p[e],
                    "cf": cfe,
                }
            )
        res = run_bass_kernel_spmd(
            nc, in_maps, core_ids=list(range(NCORES)), **RUN_KWARGS
        )
        LAST_RESULTS.append(res)
        for e in range(n_exp):
            idx_r = idx_r_per_e[e]
            if len(idx_r):
                yt = res.results[e]["yt"]  # [H, NCAP], already coeff-scaled
                out[idx_r, :] += yt[:, : len(idx_r)].T
    return out


# revision 9
# speedup vs baseline: 1.0882x; 1.0221x over previous
"""Trainium2 Bass kernel for BatchedExpertMoEDispatch.

Strategy (expert-parallel, sparse dispatch, bf16 compute):
  - Host computes the routing table from (expert_ids, expert_weights):
    for each expert e the unique token list idx_e and combined coefficient
    coeff_e (duplicate (token, expert) slots merge by summing weights).
  - The token groups are "all-to-all"ed host-side (full-I/O contract): core e
    receives its expert's tokens and weights pre-packed in partition-major
    bf16 layouts so every DMA is a straight contiguous copy (2-4KB
    per-partition lines; the fp32 rearrange loads of the previous version
    were 512B-descriptor-bound and stalled startup by ~13us).
  - Each core runs the full FFN for its expert on its tokens:
        gT = Wg.T @ xT ; uT = Wu.T @ xT          (bf16 matmul, fp32 PSUM)
        hT = silu(gT) * uT                        (ACT Silu + DVE, bf16 out)
        yT = Wd.T @ hT                            (bf16 matmul, fp32 PSUM)
        outT = yT * coeff (broadcast over partitions)
    bf16 matmuls run at the same 1 row/cycle as float32r but qualify for
    fast-weight-load + background LDWEIGHTS pull-ahead, so the per-matmul
    weight-load overhead of the fp32r version (~12%) disappears.  Stationary
    weight tiles are reused across both token chunks (chunk-inner loop) to
    halve LDWEIGHTS traffic.
  - Host scatter-adds each core's outT back: out[idx_e] += outT[:, :n_e].T.

Capacity: NCAP tokens/core/round.  If any expert has more assigned tokens,
the same compiled program runs additional rounds on the remainder.
"""

import os
import sys

import numpy as np
import ml_dtypes

for _p in ("/opt/trn_rl_repo", "/root/.axon_site/_ro/trn_rl_repo"):
    if os.path.isdir(_p) and _p not in sys.path:
        sys.path.append(_p)

import concourse.bacc as bacc
import concourse.mybir as mybir
import concourse.tile as tile
from concourse.bass_utils import run_bass_kernel_spmd

# Problem shapes (hardcoded per contract).
T, H, F, E, K = 4096, 1024, 2048, 8, 2
NCORES = 8
CKS = [512, 472]     # token chunks (PSUM bank = 512 fp32)
NCAP = sum(CKS)      # token capacity per core per round (>= seed-wise max)
COFF = [0, 512]      # chunk offsets
KH = H // 128        # 8  k-tiles over H
KF = F // 128        # 16 k-tiles over F
FP32 = mybir.dt.float32
BF16 = mybir.dt.bfloat16
NPBF16 = ml_dtypes.bfloat16
MUL = mybir.AluOpType.mult

_PROGRAM = None

# Extra kwargs for run_bass_kernel_spmd — test harness pokes this to enable
# tracing; the grader path leaves it empty.
RUN_KWARGS: dict = {}
LAST_RESULTS = []


def build_program():
    """Build + compile the per-core SPMD FFN program (shared by all cores)."""
    nc = bacc.Bacc(
        "TRN2", target_bir_lowering=False, debug=False, num_devices=NCORES
    )
    # Packed layouts (host-side prep):
    #   xp[p, k*NCAP+t] = x[idx[t], k*128+p]
    #   wg/wu[p, f*1024 + k*128 + m] = W[k*128+p, f*128+m]
    #   wd[p, j*2048 + kf*128 + m]   = Wd[kf*128+p, j*128+m]
    xp_d = nc.dram_tensor("xp", [128, KH * NCAP], BF16, kind="ExternalInput")
    wg_d = nc.dram_tensor("wg", [128, KF * KH * 128], BF16, kind="ExternalInput")
    wu_d = nc.dram_tensor("wu", [128, KF * KH * 128], BF16, kind="ExternalInput")
    wd_d = nc.dram_tensor("wd", [128, KH * KF * 128], BF16, kind="ExternalInput")
    cf_d = nc.dram_tensor("cf", [1, NCAP], FP32, kind="ExternalInput")
    yt_d = nc.dram_tensor("yt", [H, NCAP], FP32, kind="ExternalOutput")

    with tile.TileContext(nc) as tc:
        from contextlib import ExitStack

        with ExitStack() as ctx:
            xk_pool = ctx.enter_context(tc.tile_pool(name="xk", bufs=KH))
            wg_pool = ctx.enter_context(tc.tile_pool(name="wg", bufs=KF))
            wu_pool = ctx.enter_context(tc.tile_pool(name="wu", bufs=KF))
            wd_pool = ctx.enter_context(tc.tile_pool(name="wd", bufs=KH))
            ht_pool = ctx.enter_context(tc.tile_pool(name="ht", bufs=KF))
            cf_pool = ctx.enter_context(tc.tile_pool(name="cf", bufs=1))
            sl_pool = ctx.enter_context(tc.tile_pool(name="sl", bufs=4))
            ob_pool = ctx.enter_context(tc.tile_pool(name="ob", bufs=4))
            pg_pool = ctx.enter_context(tc.tile_pool(name="pg", bufs=2, space="PSUM"))
            pu_pool = ctx.enter_context(tc.tile_pool(name="pu", bufs=2, space="PSUM"))
            # 4 bufs so consecutive j-tiles never wait on the DVE drain of the
            # previous one (2 bufs cost ~580ns per j boundary).
            py_pool = ctx.enter_context(tc.tile_pool(name="py", bufs=4, space="PSUM"))

            # Every engine sequencer is blocked by the framework entry until
            # ~6.3us, and each dma_start burns ~640ns of sequencer issue time.
            # Spread the critical first transfers across five queues so they
            # issue in parallel instead of serially on sync (serial issue cost
            # the previous version ~5us of startup).
            xks = {}

            def load_xk(k, eng):
                t = xk_pool.tile([128, NCAP], BF16, tag="xk", name=f"xk{k}")
                eng.dma_start(t[:], xp_d.ap()[:, k * NCAP : (k + 1) * NCAP])
                xks[k] = t

            wgwu = {}

            def load_wgwu(f):
                wgt = wg_pool.tile([128, KH * 128], BF16, tag="wg", name=f"wg{f}")
                wut = wu_pool.tile([128, KH * 128], BF16, tag="wu", name=f"wu{f}")
                nc.sync.dma_start(
                    wgt[:], wg_d.ap()[:, f * KH * 128 : (f + 1) * KH * 128]
                )
                nc.sync.dma_start(
                    wut[:], wu_d.ap()[:, f * KH * 128 : (f + 1) * KH * 128]
                )
                wgwu[f] = (wgt, wut)

            wds = {}

            def load_wd(j):
                t = wd_pool.tile([128, KF * 128], BF16, tag="wd", name=f"wd{j}")
                nc.sync.dma_start(t[:], wd_d.ap()[:, j * KF * 128 : (j + 1) * KF * 128])
                wds[j] = t

            # Only SP (sync), Activation (scalar) and gpsimd can start DMAs.
            load_xk(0, nc.gpsimd)
            load_wgwu(0)               # wg0 + wu0 on sync
            load_xk(1, nc.gpsimd)
            load_xk(2, nc.scalar)
            load_xk(3, nc.scalar)
            # coeff broadcast: only needed in phase 2
            cf_t = cf_pool.tile([128, NCAP], FP32, tag="cf")
            nc.gpsimd.dma_start(cf_t[:], cf_d.ap().partition_broadcast(128))
            load_xk(4, nc.scalar)
            load_xk(5, nc.scalar)
            load_xk(6, nc.gpsimd)
            load_xk(7, nc.gpsimd)
            load_wgwu(1)

            # Phase 1: hT[f] = silu(Wg[:,f].T @ xT) * (Wu[:,f].T @ xT)
            hts = []
            for f in range(KF):
                if f not in wgwu:
                    load_wgwu(f)
                # prefetch a couple of f-columns ahead; down weights at the end
                pf = f + 2
                if pf < KF and pf not in wgwu:
                    load_wgwu(pf)
                if f == KF - 1:
                    for j in range(KH):
                        load_wd(j)
                wgt, wut = wgwu[f]
                ht = ht_pool.tile([128, NCAP], BF16, tag="ht")
                pgs, pus = [], []
                for ci in range(2):
                    pgs.append(
                        pg_pool.tile(
                            [128, CKS[ci]], FP32, tag="pg", name=f"pg{ci}"
                        )
                    )
                    pus.append(
                        pu_pool.tile(
                            [128, CKS[ci]], FP32, tag="pu", name=f"pu{ci}"
                        )
                    )
                if f == 0:
                    # k-outer: consume each x k-slice for gate AND up before
                    # needing the next — halves the startup DMA arrival rate
                    # the first accumulation pass demands.
                    order = [
                        (dsts, w, k)
                        for k in range(KH)
                        for dsts, w in ((pgs, wgt), (pus, wut))
                    ]
                else:
                    order = [
                        (dsts, w, k)
                        for dsts, w in ((pgs, wgt), (pus, wut))
                        for k in range(KH)
                    ]
                for dsts, w, k in order:
                    for ci in range(2):
                        cs = COFF[ci]
                        ck = CKS[ci]
                        nc.tensor.matmul(
                            dsts[ci][:],
                            w[:, k * 128 : (k + 1) * 128],
                            xks[k][:, cs : cs + ck],
                            start=(k == 0),
                            stop=(k == KH - 1),
                        )
                for ci in range(2):
                    cs, ck = COFF[ci], CKS[ci]
                    sl = sl_pool.tile([128, ck], FP32, tag="sl")
                    nc.scalar.activation(
                        sl[:], pgs[ci][:], mybir.ActivationFunctionType.Silu
                    )
                    nc.vector.tensor_tensor(
                        ht[:, cs : cs + ck], sl[:], pus[ci][:], MUL
                    )
                hts.append(ht)

            # Phase 2: yT[j] = (Wd[:,j].T @ hT) * coeff
            for j in range(KH):
                wdt = wds[j]
                pys = []
                for ci in range(2):
                    pys.append(
                        py_pool.tile(
                            [128, CKS[ci]], FP32, tag="py", name=f"py{ci}"
                        )
                    )
                for kf in range(KF):
                    for ci in range(2):
                        cs, ck = COFF[ci], CKS[ci]
                        nc.tensor.matmul(
                            pys[ci][:],
                            wdt[:, kf * 128 : (kf + 1) * 128],
                            hts[kf][:, cs : cs + ck],
                            start=(kf == 0),
                            stop=(kf == KF - 1),
                        )
                for ci in range(2):
                    cs, ck = COFF[ci], CKS[ci]
                    # Final unit: split the coeff-mul + store into halves on
                    # two different queues so the issue cost (~640ns per
                    # dma_start) of the last stores is paid in parallel.
                    nsplit = 2 if (j == KH - 1 and ci == 1) else 1
                    ob = ob_pool.tile([128, ck], FP32, tag="ob")
                    hw = ck // nsplit
                    st_engs = (nc.scalar, nc.sync)
                    for s in range(nsplit):
                        lo, hi = s * hw, (s + 1) * hw if s < nsplit - 1 else ck
                        nc.vector.tensor_tensor(
                            ob[:, lo:hi],
                            pys[ci][:, lo:hi],
                            cf_t[:, cs + lo : cs + hi],
                            MUL,
                        )
                        st_engs[s].dma_start(
                            yt_d.ap()[j * 128 : (j + 1) * 128, cs + lo : cs + hi],
                            ob[:, lo:hi],
                        )

    nc.compile()
    return nc


def _get_program():
    global _PROGRAM
    if _PROGRAM is None:
        _PROGRAM = build_program()
    return _PROGRAM


def _pack_weights(gate_weights, up_weights, down_weights):
    """Pre-pack per-expert weights into partition-major bf16 DMA images."""
    wg_p, wu_p, wd_p = [], [], []
    for e in range(NCORES):
        wg = np.ascontiguousarray(
            gate_weights[e]
            .astype(NPBF16)
            .reshape(KH, 128, KF, 128)
            .transpose(1, 2, 0, 3)
            .reshape(128, KF * KH * 128)
        )
        wu = np.ascontiguousarray(
            up_weights[e]
            .astype(NPBF16)
            .reshape(KH, 128, KF, 128)
            .transpose(1, 2, 0, 3)
            .reshape(128, KF * KH * 128)
        )
        wd = np.ascontiguousarray(
            down_weights[e]
            .astype(NPBF16)
            .reshape(KF, 128, KH, 128)
            .transpose(1, 2, 0, 3)
            .reshape(128, KH * KF * 128)
        )
        wg_p.append(wg)
        wu_p.append(wu)
        wd_p.append(wd)
    return wg_p, wu_p, wd_p


def kernel(x, expert_ids, expert_weights, gate_weights, up_weights, down_weights):
    x = np.ascontiguousarray(np.asarray(x, dtype=np.float32))
    expert_ids = np.asarray(expert_ids)
    expert_weights = np.asarray(expert_weights, dtype=np.float32)
    gate_weights = np.asarray(gate_weights, dtype=np.float32)
    up_weights = np.asarray(up_weights, dtype=np.float32)
    down_weights = np.asarray(down_weights, dtype=np.float32)

    t_dim, h_dim = x.shape
    n_exp = gate_weights.shape[0]
    assert h_dim == H and gate_weights.shape[1:] == (H, F), (
        "program compiled for H=1024, F=2048"
    )
    assert n_exp == NCORES, "expert-parallel mapping assumes E == 8 cores"

    # Routing table: per-token combined coefficient per expert.
    coeff = np.zeros((t_dim, n_exp), np.float32)
    rows = np.arange(t_dim)
    for k in range(expert_ids.shape[1]):
        np.add.at(coeff, (rows, expert_ids[:, k]), expert_weights[:, k])

    idx_per_e = [np.nonzero(coeff[:, e])[0] for e in range(n_exp)]
    rounds = max(1, max((len(i) + NCAP - 1) // NCAP for i in idx_per_e))

    wg_p, wu_p, wd_p = _pack_weights(gate_weights, up_weights, down_weights)
    x16 = x.astype(NPBF16)
    nc = _get_program()

    out = np.zeros((t_dim, h_dim), np.float32)
    LAST_RESULTS.clear()
    for r in range(rounds):
        in_maps = []
        idx_r_per_e = []
        for e in range(n_exp):
            idx_r = idx_per_e[e][r * NCAP : (r + 1) * NCAP]
            idx_r_per_e.append(idx_r)
            xpe = np.zeros((128, KH, NCAP), NPBF16)
            cfe = np.zeros((1, NCAP), np.float32)
            if len(idx_r):
                # [p, k, t] = x[idx[t], k*128+p]
                xpe[:, :, : len(idx_r)] = x16[idx_r].reshape(
                    len(idx_r), KH, 128
                ).transpose(2, 1, 0)
                cfe[0, : len(idx_r)] = coeff[idx_r, e]
            in_maps.append(
                {
                    "xp": xpe.reshape(128, KH * NCAP),
                    "wg": wg_p[e],
                    "wu": wu_p[e],
                    "wd": wd_p[e],
                    "cf": cfe,
                }
            )
        res = run_bass_kernel_spmd(
            nc, in_maps, core_ids=list(range(NCORES)), **RUN_KWARGS
        )
        LAST_RESULTS.append(res)
        for e in range(n_exp):
            idx_r = idx_r_per_e[e]
            if len(idx_r):
                yt = res.results[e]["yt"]  # [H, NCAP], already coeff-scaled
                out[idx_r, :] += yt[:, : len(idx_r)].T
    return out


# revision 11
# speedup vs baseline: 1.0907x; 1.0023x over previous
"""Trainium2 Bass kernel for BatchedExpertMoEDispatch.

Strategy (expert-parallel, sparse dispatch, bf16 compute):
  - Host computes the routing table from (expert_ids, expert_weights):
    for each expert e the unique token list idx_e and combined coefficient
    coeff_e (duplicate (token, expert) slots merge by summing weights).
  - The token groups are "all-to-all"ed host-side (full-I/O contract): core e
    receives its expert's tokens and weights pre-packed in partition-major
    bf16 layouts so every DMA is a straight contiguous copy (2-4KB
    per-partition lines; the fp32 rearrange loads of the previous version
    were 512B-descriptor-bound and stalled startup by ~13us).
  - Each core runs the full FFN for its expert on its tokens:
        gT = Wg.T @ xT ; uT = Wu.T @ xT          (bf16 matmul, fp32 PSUM)
        hT = silu(gT) * uT                        (ACT Silu + DVE, bf16 out)
        yT = Wd.T @ hT                            (bf16 matmul, fp32 PSUM)
        outT = yT * coeff (broadcast over partitions)
    bf16 matmuls run at the same 1 row/cycle as float32r but qualify for
    fast-weight-load + background LDWEIGHTS pull-ahead, so the per-matmul
    weight-load overhead of the fp32r version (~12%) disappears.  Stationary
    weight tiles are reused across both token chunks (chunk-inner loop) to
    halve LDWEIGHTS traffic.
  - Host scatter-adds each core's outT back: out[idx_e] += outT[:, :n_e].T.

Capacity: NCAP tokens/core/round.  If any expert has more assigned tokens,
the same compiled program runs additional rounds on the remainder.
"""

import os
import sys

import numpy as np
import ml_dtypes

for _p in ("/opt/trn_rl_repo", "/root/.axon_site/_ro/trn_rl_repo"):
    if os.path.isdir(_p) and _p not in sys.path:
        sys.path.append(_p)

import concourse.bacc as bacc
import concourse.mybir as mybir
import concourse.tile as tile
from concourse.bass_utils import run_bass_kernel_spmd

# Problem shapes (hardcoded per contract).
T, H, F, E, K = 4096, 1024, 2048, 8, 2
NCORES = 8
CKS = [512, 472]     # token chunks (PSUM bank = 512 fp32)
NCAP = sum(CKS)      # token capacity per core per round (>= seed-wise max)
COFF = [0, 512]      # chunk offsets
KH = H // 128        # 8  k-tiles over H
KF = F // 128        # 16 k-tiles over F
FP32 = mybir.dt.float32
BF16 = mybir.dt.bfloat16
NPBF16 = ml_dtypes.bfloat16
MUL = mybir.AluOpType.mult

_PROGRAM = None

# Extra kwargs for run_bass_kernel_spmd — test harness pokes this to enable
# tracing; the grader path leaves it empty.
RUN_KWARGS: dict = {}
LAST_RESULTS = []


def build_program():
    """Build + compile the per-core SPMD FFN program (shared by all cores)."""
    nc = bacc.Bacc(
        "TRN2", target_bir_lowering=False, debug=False, num_devices=NCORES
    )
    # Packed layouts (host-side prep):
    #   xp[p, k*NCAP+t] = x[idx[t], k*128+p]
    #   wg/wu[p, f*1024 + k*128 + m] = W[k*128+p, f*128+m]
    #   wd[p, j*2048 + kf*128 + m]   = Wd[kf*128+p, j*128+m]
    xp_d = nc.dram_tensor("xp", [128, KH * NCAP], BF16, kind="ExternalInput")
    wg_d = nc.dram_tensor("wg", [128, KF * KH * 128], BF16, kind="ExternalInput")
    wu_d = nc.dram_tensor("wu", [128, KF * KH * 128], BF16, kind="ExternalInput")
    wd_d = nc.dram_tensor("wd", [128, KH * KF * 128], BF16, kind="ExternalInput")
    cf_d = nc.dram_tensor("cf", [1, NCAP], FP32, kind="ExternalInput")
    yt_d = nc.dram_tensor("yt", [H, NCAP], FP32, kind="ExternalOutput")

    with tile.TileContext(nc) as tc:
        from contextlib import ExitStack

        with ExitStack() as ctx:
            xk_pool = ctx.enter_context(tc.tile_pool(name="xk", bufs=KH))
            wg_pool = ctx.enter_context(tc.tile_pool(name="wg", bufs=KF))
            wu_pool = ctx.enter_context(tc.tile_pool(name="wu", bufs=KF))
            wd_pool = ctx.enter_context(tc.tile_pool(name="wd", bufs=KH))
            ht_pool = ctx.enter_context(tc.tile_pool(name="ht", bufs=KF))
            cf_pool = ctx.enter_context(tc.tile_pool(name="cf", bufs=1))
            sl_pool = ctx.enter_context(tc.tile_pool(name="sl", bufs=4))
            ob_pool = ctx.enter_context(tc.tile_pool(name="ob", bufs=4))
            pg_pool = ctx.enter_context(tc.tile_pool(name="pg", bufs=2, space="PSUM"))
            pu_pool = ctx.enter_context(tc.tile_pool(name="pu", bufs=2, space="PSUM"))
            # 4 bufs so consecutive j-tiles never wait on the DVE drain of the
            # previous one (2 bufs cost ~580ns per j boundary).
            py_pool = ctx.enter_context(tc.tile_pool(name="py", bufs=4, space="PSUM"))

            # Every engine sequencer is blocked by the framework entry until
            # ~6.3us, and each dma_start burns ~640ns of sequencer issue time.
            # Spread the critical first transfers across five queues so they
            # issue in parallel instead of serially on sync (serial issue cost
            # the previous version ~5us of startup).
            xks = {}

            def load_xk(k, eng):
                t = xk_pool.tile([128, NCAP], BF16, tag="xk", name=f"xk{k}")
                eng.dma_start(t[:], xp_d.ap()[:, k * NCAP : (k + 1) * NCAP])
                xks[k] = t

            wgwu = {}

            def load_wgwu(f):
                wgt = wg_pool.tile([128, KH * 128], BF16, tag="wg", name=f"wg{f}")
                wut = wu_pool.tile([128, KH * 128], BF16, tag="wu", name=f"wu{f}")
                nc.sync.dma_start(
                    wgt[:], wg_d.ap()[:, f * KH * 128 : (f + 1) * KH * 128]
                )
                nc.sync.dma_start(
                    wut[:], wu_d.ap()[:, f * KH * 128 : (f + 1) * KH * 128]
                )
                wgwu[f] = (wgt, wut)

            wds = {}

            def load_wd(j):
                t = wd_pool.tile([128, KF * 128], BF16, tag="wd", name=f"wd{j}")
                nc.sync.dma_start(t[:], wd_d.ap()[:, j * KF * 128 : (j + 1) * KF * 128])
                wds[j] = t

            # Only SP (sync), Activation (scalar) and gpsimd can start DMAs.
            load_xk(0, nc.gpsimd)
            load_wgwu(0)               # wg0 + wu0 on sync
            load_xk(1, nc.gpsimd)
            load_xk(2, nc.scalar)
            load_xk(3, nc.scalar)
            load_xk(4, nc.scalar)
            load_xk(5, nc.scalar)
            load_xk(6, nc.gpsimd)
            load_xk(7, nc.gpsimd)
            # coeff broadcast: only needed in phase 2, so issue it after the
            # startup-critical x slices.
            cf_t = cf_pool.tile([128, NCAP], FP32, tag="cf")
            nc.gpsimd.dma_start(cf_t[:], cf_d.ap().partition_broadcast(128))
            load_wgwu(1)

            # PE warm-up: the tensor engine p-state/HAM ramp costs ~5us at
            # reduced clock on the first real matmuls.  Burn the ramp on dummy
            # matmuls over a zeroed scratch tile while the startup DMAs are
            # still in flight (PE queue is idle until ~12us otherwise).
            wm = sl_pool.tile([128, 512], BF16, tag="sl", name="wm")
            nc.vector.memset(wm[:], 0)
            pw = py_pool.tile([128, 512], FP32, tag="py", name="pw")
            for _ in range(10):
                nc.tensor.matmul(
                    pw[:], wm[:, 0:128], wm[:, 0:512], start=True, stop=True
                )

            # Phase 1: hT[f] = silu(Wg[:,f].T @ xT) * (Wu[:,f].T @ xT)
            hts = []
            for f in range(KF):
                if f not in wgwu:
                    load_wgwu(f)
                # prefetch a couple of f-columns ahead; down weights at the end
                pf = f + 2
                if pf < KF and pf not in wgwu:
                    load_wgwu(pf)
                if f == KF - 1:
                    for j in range(KH):
                        load_wd(j)
                wgt, wut = wgwu[f]
                ht = ht_pool.tile([128, NCAP], BF16, tag="ht")
                pgs, pus = [], []
                for ci in range(2):
                    pgs.append(
                        pg_pool.tile(
                            [128, CKS[ci]], FP32, tag="pg", name=f"pg{ci}"
                        )
                    )
                    pus.append(
                        pu_pool.tile(
                            [128, CKS[ci]], FP32, tag="pu", name=f"pu{ci}"
                        )
                    )
                if f == 0:
                    # k-outer: consume each x k-slice for gate AND up before
                    # needing the next — halves the startup DMA arrival rate
                    # the first accumulation pass demands.
                    order = [
                        (dsts, w, k)
                        for k in range(KH)
                        for dsts, w in ((pgs, wgt), (pus, wut))
                    ]
                else:
                    order = [
                        (dsts, w, k)
                        for dsts, w in ((pgs, wgt), (pus, wut))
                        for k in range(KH)
                    ]
                for dsts, w, k in order:
                    for ci in range(2):
                        cs = COFF[ci]
                        ck = CKS[ci]
                        nc.tensor.matmul(
                            dsts[ci][:],
                            w[:, k * 128 : (k + 1) * 128],
                            xks[k][:, cs : cs + ck],
                            start=(k == 0),
                            stop=(k == KH - 1),
                        )
                for ci in range(2):
                    cs, ck = COFF[ci], CKS[ci]
                    sl = sl_pool.tile([128, ck], FP32, tag="sl")
                    nc.scalar.activation(
                        sl[:], pgs[ci][:], mybir.ActivationFunctionType.Silu
                    )
                    nc.vector.tensor_tensor(
                        ht[:, cs : cs + ck], sl[:], pus[ci][:], MUL
                    )
                hts.append(ht)

            # Phase 2: yT[j] = (Wd[:,j].T @ hT) * coeff
            for j in range(KH):
                wdt = wds[j]
                pys = []
                for ci in range(2):
                    pys.append(
                        py_pool.tile(
                            [128, CKS[ci]], FP32, tag="py", name=f"py{ci}"
                        )
                    )
                for kf in range(KF):
                    for ci in range(2):
                        cs, ck = COFF[ci], CKS[ci]
                        nc.tensor.matmul(
                            pys[ci][:],
                            wdt[:, kf * 128 : (kf + 1) * 128],
                            hts[kf][:, cs : cs + ck],
                            start=(kf == 0),
                            stop=(kf == KF - 1),
                        )
                for ci in range(2):
                    cs, ck = COFF[ci], CKS[ci]
                    # Final unit: split the coeff-mul + store into halves on
                    # two different queues so the issue cost (~640ns per
                    # dma_start) of the last stores is paid in parallel.
                    nsplit = 2 if j == KH - 1 else 1
                    ob = ob_pool.tile([128, ck], FP32, tag="ob")
                    hw = ck // nsplit
                    st_engs = (nc.scalar, nc.sync)
                    for s in range(nsplit):
                        lo, hi = s * hw, (s + 1) * hw if s < nsplit - 1 else ck
                        nc.vector.tensor_tensor(
                            ob[:, lo:hi],
                            pys[ci][:, lo:hi],
                            cf_t[:, cs + lo : cs + hi],
                            MUL,
                        )
                        st_engs[s].dma_start(
                            yt_d.ap()[j * 128 : (j + 1) * 128, cs + lo : cs + hi],
                            ob[:, lo:hi],
                        )

    nc.compile()
    return nc


def _get_program():
    global _PROGRAM
    if _PROGRAM is None:
        _PROGRAM = build_program()
    return _PROGRAM


def _pack_weights(gate_weights, up_weights, down_weights):
    """Pre-pack per-expert weights into partition-major bf16 DMA images."""
    wg_p, wu_p, wd_p = [], [], []
    for e in range(NCORES):
        wg = np.ascontiguousarray(
            gate_weights[e]
            .astype(NPBF16)
            .reshape(KH, 128, KF, 128)
            .transpose(1, 2, 0, 3)
            .reshape(128, KF * KH * 128)
        )
        wu = np.ascontiguousarray(
            up_weights[e]
            .astype(NPBF16)
            .reshape(KH, 128, KF, 128)
            .transpose(1, 2, 0, 3)
            .reshape(128, KF * KH * 128)
        )
        wd = np.ascontiguousarray(
            down_weights[e]
            .astype(NPBF16)
            .reshape(KF, 128, KH, 128)
            .transpose(1, 2, 0, 3)
            .reshape(128, KH * KF * 128)
        )
        wg_p.append(wg)
        wu_p.append(wu)
        wd_p.append(wd)
    return wg_p, wu_p, wd_p


def kernel(x, expert_ids, expert_weights, gate_weights, up_weights, down_weights):
    x = np.ascontiguousarray(np.asarray(x, dtype=np.float32))
    expert_ids = np.asarray(expert_ids)
    expert_weights = np.asarray(expert_weights, dtype=np.float32)
    gate_weights = np.asarray(gate_weights, dtype=np.float32)
    up_weights = np.asarray(up_weights, dtype=np.float32)
    down_weights = np.asarray(down_weights, dtype=np.float32)

    t_dim, h_dim = x.shape
    n_exp = gate_weights.shape[0]
    assert h_dim == H and gate_weights.shape[1:] == (H, F), (
        "program compiled for H=1024, F=2048"
    )
    assert n_exp == NCORES, "expert-parallel mapping assumes E == 8 cores"

    # Routing table: per-token combined coefficient per expert.
    coeff = np.zeros((t_dim, n_exp), np.float32)
    rows = np.arange(t_dim)
    for k in range(expert_ids.shape[1]):
        np.add.at(coeff, (rows, expert_ids[:, k]), expert_weights[:, k])

    idx_per_e = [np.nonzero(coeff[:, e])[0] for e in range(n_exp)]
    rounds = max(1, max((len(i) + NCAP - 1) // NCAP for i in idx_per_e))

    wg_p, wu_p, wd_p = _pack_weights(gate_weights, up_weights, down_weights)
    x16 = x.astype(NPBF16)
    nc = _get_program()

    out = np.zeros((t_dim, h_dim), np.float32)
    LAST_RESULTS.clear()
    for r in range(rounds):
        in_maps = []
        idx_r_per_e = []
        for e in range(n_exp):
            idx_r = idx_per_e[e][r * NCAP : (r + 1) * NCAP]
            idx_r_per_e.append(idx_r)
            xpe = np.zeros((128, KH, NCAP), NPBF16)
            cfe = np.zeros((1, NCAP), np.float32)
            if len(idx_r):
                # [p, k, t] = x[idx[t], k*128+p]
                xpe[:, :, : len(idx_r)] = x16[idx_r].reshape(
                    len(idx_r), KH, 128
                ).transpose(2, 1, 0)
                cfe[0, : len(idx_r)] = coeff[idx_r, e]
            in_maps.append(
                {
                    "xp": xpe.reshape(128, KH * NCAP),
                    "wg": wg_p[e],
                    "wu": wu_p[e],
                    "wd": wd_p[e],
                    "cf": cfe,
                }
            )
        res = run_bass_kernel_spmd(
            nc, in_maps, core_ids=list(range(NCORES)), **RUN_KWARGS
        )
        LAST_RESULTS.append(res)
        for e in range(n_exp):
            idx_r = idx_r_per_e[e]
            if len(idx_r):
                yt = res.results[e]["yt"]  # [H, NCAP], already coeff-scaled
                out[idx_r, :] += yt[:, : len(idx_r)].T
    return out


# revision 17
# speedup vs baseline: 1.0921x; 1.0013x over previous
"""Trainium2 Bass kernel for BatchedExpertMoEDispatch.

Strategy (expert-parallel, sparse dispatch, bf16 compute):
  - Host computes the routing table from (expert_ids, expert_weights):
    for each expert e the unique token list idx_e and combined coefficient
    coeff_e (duplicate (token, expert) slots merge by summing weights).
  - The token groups are "all-to-all"ed host-side (full-I/O contract): core e
    receives its expert's tokens and weights pre-packed in partition-major
    bf16 layouts so every DMA is a straight contiguous copy (2-4KB
    per-partition lines; the fp32 rearrange loads of the previous version
    were 512B-descriptor-bound and stalled startup by ~13us).
  - Each core runs the full FFN for its expert on its tokens:
        gT = Wg.T @ xT ; uT = Wu.T @ xT          (bf16 matmul, fp32 PSUM)
        hT = silu(gT) * uT                        (ACT Silu + DVE, bf16 out)
        yT = Wd.T @ hT                            (bf16 matmul, fp32 PSUM)
        outT = yT * coeff (broadcast over partitions)
    bf16 matmuls run at the same 1 row/cycle as float32r but qualify for
    fast-weight-load + background LDWEIGHTS pull-ahead, so the per-matmul
    weight-load overhead of the fp32r version (~12%) disappears.  Stationary
    weight tiles are reused across both token chunks (chunk-inner loop) to
    halve LDWEIGHTS traffic.
  - Host scatter-adds each core's outT back: out[idx_e] += outT[:, :n_e].T.

Capacity: NCAP tokens/core/round.  If any expert has more assigned tokens,
the same compiled program runs additional rounds on the remainder.
"""

import os
import sys

import numpy as np
import ml_dtypes

for _p in ("/opt/trn_rl_repo", "/root/.axon_site/_ro/trn_rl_repo"):
    if os.path.isdir(_p) and _p not in sys.path:
        sys.path.append(_p)

import concourse.bacc as bacc
import concourse.mybir as mybir
import concourse.tile as tile
from concourse.bass_utils import run_bass_kernel_spmd

# Problem shapes (hardcoded per contract).
T, H, F, E, K = 4096, 1024, 2048, 8, 2
NCORES = 8
CKS = [512, 472]     # token chunks (PSUM bank = 512 fp32)
NCAP = sum(CKS)      # token capacity per core per round (>= seed-wise max)
COFF = [0, 512]      # chunk offsets
KH = H // 128        # 8  k-tiles over H
KF = F // 128        # 16 k-tiles over F
FP32 = mybir.dt.float32
BF16 = mybir.dt.bfloat16
NPBF16 = ml_dtypes.bfloat16
MUL = mybir.AluOpType.mult

_PROGRAM = None

# Extra kwargs for run_bass_kernel_spmd — test harness pokes this to enable
# tracing; the grader path leaves it empty.
RUN_KWARGS: dict = {}
LAST_RESULTS = []


def build_program():
    """Build + compile the per-core SPMD FFN program (shared by all cores)."""
    nc = bacc.Bacc(
        "TRN2", target_bir_lowering=False, debug=False, num_devices=NCORES
    )
    # Packed layouts (host-side prep):
    #   xp[p, k*NCAP+t] = x[idx[t], k*128+p]
    #   wg/wu[p, f*1024 + k*128 + m] = W[k*128+p, f*128+m]
    #   wd[p, j*2048 + kf*128 + m]   = Wd[kf*128+p, j*128+m]
    xp_d = nc.dram_tensor("xp", [128, KH * NCAP], BF16, kind="ExternalInput")
    wg_d = nc.dram_tensor("wg", [128, KF * KH * 128], BF16, kind="ExternalInput")
    wu_d = nc.dram_tensor("wu", [128, KF * KH * 128], BF16, kind="ExternalInput")
    wd_d = nc.dram_tensor("wd", [128, KH * KF * 128], BF16, kind="ExternalInput")
    cf_d = nc.dram_tensor("cf", [1, NCAP], FP32, kind="ExternalInput")
    yt_d = nc.dram_tensor("yt", [H, NCAP], FP32, kind="ExternalOutput")

    with tile.TileContext(nc) as tc:
        from contextlib import ExitStack

        with ExitStack() as ctx:
            xk_pool = ctx.enter_context(tc.tile_pool(name="xk", bufs=KH))
            wg_pool = ctx.enter_context(tc.tile_pool(name="wg", bufs=KF + 1))
            wu_pool = ctx.enter_context(tc.tile_pool(name="wu", bufs=KF + 1))
            wd_pool = ctx.enter_context(tc.tile_pool(name="wd", bufs=KH))
            ht_pool = ctx.enter_context(tc.tile_pool(name="ht", bufs=KF))
            cf_pool = ctx.enter_context(tc.tile_pool(name="cf", bufs=1))
            sl_pool = ctx.enter_context(tc.tile_pool(name="sl", bufs=4))
            ob_pool = ctx.enter_context(tc.tile_pool(name="ob", bufs=4))
            pg_pool = ctx.enter_context(tc.tile_pool(name="pg", bufs=2, space="PSUM"))
            pu_pool = ctx.enter_context(tc.tile_pool(name="pu", bufs=2, space="PSUM"))
            # 4 bufs so consecutive j-tiles never wait on the DVE drain of the
            # previous one (2 bufs cost ~580ns per j boundary).
            py_pool = ctx.enter_context(tc.tile_pool(name="py", bufs=4, space="PSUM"))

            # Every engine sequencer is blocked by the framework entry until
            # ~6.3us, and each dma_start burns ~640ns of sequencer issue time.
            # Spread the critical first transfers across five queues so they
            # issue in parallel instead of serially on sync (serial issue cost
            # the previous version ~5us of startup).
            xks = {}

            def load_xk(k, eng):
                t = xk_pool.tile([128, NCAP], BF16, tag="xk", name=f"xk{k}")
                eng.dma_start(t[:], xp_d.ap()[:, k * NCAP : (k + 1) * NCAP])
                xks[k] = t

            # wgwu[f] = (list of (tile, k_lo, k_hi), same for wu); f0 is loaded
            # as k-halves so the first matmul only waits for 128KB of weights.
            wgwu = {}

            def load_wgwu(f):
                wgt = wg_pool.tile([128, KH * 128], BF16, tag="wg", name=f"wg{f}")
                wut = wu_pool.tile([128, KH * 128], BF16, tag="wu", name=f"wu{f}")
                nc.sync.dma_start(
                    wgt[:], wg_d.ap()[:, f * KH * 128 : (f + 1) * KH * 128]
                )
                nc.sync.dma_start(
                    wut[:], wu_d.ap()[:, f * KH * 128 : (f + 1) * KH * 128]
                )
                wgwu[f] = ([(wgt, 0, KH)], [(wut, 0, KH)])

            def wk_ap(parts, k):
                for t, k_lo, k_hi in parts:
                    if k_lo <= k < k_hi:
                        return t[:, (k - k_lo) * 128 : (k - k_lo + 1) * 128]
                raise AssertionError

            wds = {}

            def load_wd(j):
                t = wd_pool.tile([128, KF * 128], BF16, tag="wd", name=f"wd{j}")
                nc.sync.dma_start(t[:], wd_d.ap()[:, j * KF * 128 : (j + 1) * KF * 128])
                wds[j] = t

            # Only SP (sync), Activation (scalar) and gpsimd can start DMAs.
            # The f0 pass consumes (wg_k, wu_k, xk_k) per k at ~850ns/k, so
            # stage the arrivals to match: x slices on gpsimd+scalar, f0
            # weight k-halves interleaved g/u on sync.
            load_xk(0, nc.gpsimd)
            f0_parts = ([], [])
            for h in range(2):
                for gi, wdram in ((0, wg_d), (1, wu_d)):
                    t = (wg_pool if gi == 0 else wu_pool).tile(
                        [128, 4 * 128], BF16, tag=("wg" if gi == 0 else "wu"),
                        name=f"w{gi}h{h}",
                    )
                    nc.sync.dma_start(
                        t[:], wdram.ap()[:, h * 4 * 128 : (h + 1) * 4 * 128]
                    )
                    f0_parts[gi].append((t, h * 4, (h + 1) * 4))
            wgwu[0] = f0_parts
            load_xk(1, nc.gpsimd)
            load_xk(2, nc.scalar)
            load_xk(3, nc.scalar)
            load_xk(4, nc.scalar)
            load_xk(5, nc.scalar)
            load_xk(6, nc.gpsimd)
            load_xk(7, nc.gpsimd)
            # coeff broadcast: only needed in phase 2, so issue it after the
            # startup-critical x slices.
            cf_t = cf_pool.tile([128, NCAP], FP32, tag="cf")
            nc.gpsimd.dma_start(cf_t[:], cf_d.ap().partition_broadcast(128))
            load_wgwu(1)

            # PE warm-up: a few dummy matmuls over a zeroed scratch tile while
            # the startup DMAs are in flight, so part of the p-state/HAM clock
            # ramp burns during otherwise-idle PE time.
            wm = sl_pool.tile([128, 512], BF16, tag="sl", name="wm")
            nc.vector.memset(wm[:], 0)
            pw = py_pool.tile([128, 512], FP32, tag="py", name="pw")
            for _ in range(4):
                nc.tensor.matmul(
                    pw[:], wm[:, 0:128], wm[:, 0:512], start=True, stop=True
                )

            # Phase 1: hT[f] = silu(Wg[:,f].T @ xT) * (Wu[:,f].T @ xT)
            hts = []
            for f in range(KF):
                if f not in wgwu:
                    load_wgwu(f)
                # prefetch a couple of f-columns ahead; down weights at the end
                pf = f + 2
                if pf < KF and pf not in wgwu:
                    load_wgwu(pf)
                if f == KF - 1:
                    for j in range(KH):
                        load_wd(j)
                wg_parts, wu_parts = wgwu[f]
                ht = ht_pool.tile([128, NCAP], BF16, tag="ht")
                pgs, pus = [], []
                for ci in range(2):
                    pgs.append(
                        pg_pool.tile(
                            [128, CKS[ci]], FP32, tag="pg", name=f"pg{ci}"
                        )
                    )
                    pus.append(
                        pu_pool.tile(
                            [128, CKS[ci]], FP32, tag="pu", name=f"pu{ci}"
                        )
                    )
                if f == 0:
                    # k-outer: consume each (wg_k, wu_k, x_k) triple before
                    # needing the next — matches the startup DMA arrival rate.
                    order = [
                        (dsts, wp, k)
                        for k in range(KH)
                        for dsts, wp in ((pgs, wg_parts), (pus, wu_parts))
                    ]
                else:
                    order = [
                        (dsts, wp, k)
                        for dsts, wp in ((pgs, wg_parts), (pus, wu_parts))
                        for k in range(KH)
                    ]
                for dsts, wp, k in order:
                    for ci in range(2):
                        cs = COFF[ci]
                        ck = CKS[ci]
                        nc.tensor.matmul(
                            dsts[ci][:],
                            wk_ap(wp, k),
                            xks[k][:, cs : cs + ck],
                            start=(k == 0),
                            stop=(k == KH - 1),
                        )
                for ci in range(2):
                    cs, ck = COFF[ci], CKS[ci]
                    sl = sl_pool.tile([128, ck], FP32, tag="sl")
                    nc.scalar.activation(
                        sl[:], pgs[ci][:], mybir.ActivationFunctionType.Silu
                    )
                    nc.vector.tensor_tensor(
                        ht[:, cs : cs + ck], sl[:], pus[ci][:], MUL
                    )
                hts.append(ht)

            # Phase 2: yT[j] = (Wd[:,j].T @ hT) * coeff
            for j in range(KH):
                wdt = wds[j]
                pys = []
                for ci in range(2):
                    pys.append(
                        py_pool.tile(
                            [128, CKS[ci]], FP32, tag="py", name=f"py{ci}"
                        )
                    )
                for kf in range(KF):
                    for ci in range(2):
                        cs, ck = COFF[ci], CKS[ci]
                        nc.tensor.matmul(
                            pys[ci][:],
                            wdt[:, kf * 128 : (kf + 1) * 128],
                            hts[kf][:, cs : cs + ck],
                            start=(kf == 0),
                            stop=(kf == KF - 1),
                        )
                for ci in range(2):
                    cs, ck = COFF[ci], CKS[ci]
                    # Stores alternate between the scalar and sync queues so
                    # the ~640ns issue cost of consecutive stores overlaps
                    # (sync is idle in phase 2).
                    ob = ob_pool.tile([128, ck], FP32, tag="ob")
                    nc.vector.tensor_tensor(
                        ob[:], pys[ci][:], cf_t[:, cs : cs + ck], MUL
                    )
                    (nc.scalar if ci == 0 else nc.sync).dma_start(
                        yt_d.ap()[j * 128 : (j + 1) * 128, cs : cs + ck],
                        ob[:],
                    )

    nc.compile()
    return nc


def _get_program():
    global _PROGRAM
    if _PROGRAM is None:
        _PROGRAM = build_program()
    return _PROGRAM


def _pack_weights(gate_weights, up_weights, down_weights):
    """Pre-pack per-expert weights into partition-major bf16 DMA images."""
    wg_p, wu_p, wd_p = [], [], []
    for e in range(NCORES):
        wg = np.ascontiguousarray(
            gate_weights[e]
            .astype(NPBF16)
            .reshape(KH, 128, KF, 128)
            .transpose(1, 2, 0, 3)
            .reshape(128, KF * KH * 128)
        )
        wu = np.ascontiguousarray(
            up_weights[e]
            .astype(NPBF16)
            .reshape(KH, 128, KF, 128)
            .transpose(1, 2, 0, 3)
            .reshape(128, KF * KH * 128)
        )
        wd = np.ascontiguousarray(
            down_weights[e]
            .astype(NPBF16)
            .reshape(KF, 128, KH, 128)
            .transpose(1, 2, 0, 3)
            .reshape(128, KH * KF * 128)
        )
        wg_p.append(wg)
        wu_p.append(wu)
        wd_p.append(wd)
    return wg_p, wu_p, wd_p


def kernel(x, expert_ids, expert_weights, gate_weights, up_weights, down_weights):
    x = np.ascontiguousarray(np.asarray(x, dtype=np.float32))
    expert_ids = np.asarray(expert_ids)
    expert_weights = np.asarray(expert_weights, dtype=np.float32)
    gate_weights = np.asarray(gate_weights, dtype=np.float32)
    up_weights = np.asarray(up_weights, dtype=np.float32)
    down_weights = np.asarray(down_weights, dtype=np.float32)

    t_dim, h_dim = x.shape
    n_exp = gate_weights.shape[0]
    assert h_dim == H and gate_weights.shape[1:] == (H, F), (
        "program compiled for H=1024, F=2048"
    )
    assert n_exp == NCORES, "expert-parallel mapping assumes E == 8 cores"

    # Routing table: per-token combined coefficient per expert.
    coeff = np.zeros((t_dim, n_exp), np.float32)
    rows = np.arange(t_dim)
    for k in range(expert_ids.shape[1]):
        np.add.at(coeff, (rows, expert_ids[:, k]), expert_weights[:, k])

    idx_per_e = [np.nonzero(coeff[:, e])[0] for e in range(n_exp)]
    rounds = max(1, max((len(i) + NCAP - 1) // NCAP for i in idx_per_e))

    wg_p, wu_p, wd_p = _pack_weights(gate_weights, up_weights, down_weights)
    x16 = x.astype(NPBF16)
    nc = _get_program()

    out = np.zeros((t_dim, h_dim), np.float32)
    LAST_RESULTS.clear()
    for r in range(rounds):
        in_maps = []
        idx_r_per_e = []
        for e in range(n_exp):
            idx_r = idx_per_e[e][r * NCAP : (r + 1) * NCAP]
            idx_r_per_e.append(idx_r)
            xpe = np.zeros((128, KH, NCAP), NPBF16)
            cfe = np.zeros((1, NCAP), np.float32)
            if len(idx_r):
                # [p, k, t] = x[idx[t], k*128+p]
                xpe[:, :, : len(idx_r)] = x16[idx_r].reshape(
                    len(idx_r), KH, 128
                ).transpose(2, 1, 0)
                cfe[0, : len(idx_r)] = coeff[idx_r, e]
            in_maps.append(
                {
                    "xp": xpe.reshape(128, KH * NCAP),
                    "wg": wg_p[e],
                    "wu": wu_p[e],
                    "wd": wd_p[e],
                    "cf": cfe,
                }
            )
        res = run_bass_kernel_spmd(
            nc, in_maps, core_ids=list(range(NCORES)), **RUN_KWARGS
        )
        LAST_RESULTS.append(res)
        for e in range(n_exp):
            idx_r = idx_r_per_e[e]
            if len(idx_r):
                yt = res.results[e]["yt"]  # [H, NCAP], already coeff-scaled
                out[idx_r, :] += yt[:, : len(idx_r)].T
    return out
